# revision 24
# baseline (speedup 1.0000x reference)
"""GNN message-passing (scatter_mean -> BN -> Linear -> ReLU) on 8 TRN2 cores.

Strategy (edge partition via target-node bin-packing + paired source gather):
  - Host bin-packs the 50000 target nodes into 392 groups of 128 slots so
    every group has ~equal total in-degree (~2041 edges).  Core i owns 49
    groups.  Every core runs the identical instruction stream.
  - The gather is indirect-DMA-call-rate bound (~1.5us per 128-row call), so
    the host pairs up sources per core: nodes whose edge group-signatures
    match share a 256B descriptor (two adjacent rows of a per-core permuted
    bf16 x table), so one call fetches 256 edges.  Unpaired edges use plain
    128-row calls.
  - Device, per group: gather pair tiles [128,128] (2 sub-tiles) and single
    tiles [128,64], build one-hot(target-slot) bf16 matrices on DVE, and
    accumulate sum_T[c, n] on the PE via 1-pass bf16 matmuls (psum fp32).
  - BN batch stats: per-core partial sum / sum-of-squares per channel,
    AllReduce'd across the 8 cores (2x64 floats), then folded into the
    Linear: out = relu(agg @ (a*W^T) + b2).
  - Host reassembles the full [50000, 64] output from the per-core bands.
"""

import sys
import heapq
from collections import defaultdict

import numpy as np
import ml_dtypes

for _p in ("/opt/trn_rl_repo",):
    if _p not in sys.path:
        sys.path.append(_p)

import concourse.bacc as bacc
import concourse.bass as bass
import concourse.tile as tile
import concourse.mybir as mybir
from concourse import bass_utils

N_NODES = 50000
N_EDGES = 800000
C = 64
BN_EPS = 1e-5
N_CORES = 8
N_TAB = 50048  # permuted x table rows (>= N_NODES, padded with zeros)


def plan_shard(targets, n_nodes, n_cores, groups_per_core):
    """Bin-pack nodes into (n_cores*groups_per_core) groups of 128 slots with
    ~equal total degree."""
    n_groups = n_cores * groups_per_core
    deg = np.bincount(targets, minlength=n_nodes).astype(np.int64)
    order = np.argsort(-deg, kind="stable")
    node_group = np.empty(n_nodes, np.int32)
    node_slot = np.empty(n_nodes, np.int32)
    heap = [(0, g) for g in range(n_groups)]
    heapq.heapify(heap)
    fill = np.zeros(n_groups, np.int32)
    for n in order:
        d = int(deg[n])
        while True:
            load, g = heapq.heappop(heap)
            if fill[g] < 128:
                break
        node_group[n] = g
        node_slot[n] = fill[g]
        fill[g] += 1
        if fill[g] < 128:
            heapq.heappush(heap, (load + d, g))
    return deg, node_group, node_slot


def build_tables(x_bf, sources, targets, gpc):
    """Pair/quad sources per core and build gather tables.

    Nodes may occupy up to 3 rows of the per-core table (quad/pair/single
    copies) -- bounded duplication, table stays O(N).  Returns
    (Q, P, S, ntab, x_perm[NC], idxq, tgtq, idxp, tgtp, idxs, tgts, recip,
    node_group, node_slot)."""
    deg, node_group, node_slot = plan_shard(targets, N_NODES, N_CORES, gpc)
    eg = node_group[targets]
    ecore = eg // gpc
    egl = eg % gpc
    eslot = node_slot[targets]

    def pair_nodes(sig):
        """3-pass greedy matching on {node: {g: [slots]}}; returns paired."""
        buckets = defaultdict(list)
        for node, gc in sig.items():
            key = tuple(sorted((g, len(v)) for g, v in gc.items()))
            buckets[key].append(node)
        paired = {}
        leftover = []
        for key, nodes in buckets.items():
            k = len(nodes) // 2 * 2
            for j in range(0, k, 2):
                paired[nodes[j]] = nodes[j + 1]
                paired[nodes[j + 1]] = nodes[j]
            if len(nodes) % 2:
                leftover.append(nodes[-1])
        bset = defaultdict(list)
        for node in leftover:
            bset[tuple(sorted(sig[node]))].append(node)
        leftover2 = []
        for key, nodes in bset.items():
            nodes.sort(key=lambda n: tuple(sorted(len(v) for v in sig[n].values())))
            k = len(nodes) // 2 * 2
            for j in range(0, k, 2):
                paired[nodes[j]] = nodes[j + 1]
                paired[nodes[j + 1]] = nodes[j]
            if len(nodes) % 2:
                leftover2.append(nodes[-1])
        b2 = defaultdict(list)
        for node in leftover2:
            gc = sig[node]
            gmax = max(gc.items(), key=lambda kv: len(kv[1]))[0]
            b2[gmax].append(node)
        for g, nodes in b2.items():
            nodes.sort(key=lambda n: -len(sig[n][g]))
            k = len(nodes) // 2 * 2
            for j in range(0, k, 2):
                paired[nodes[j]] = nodes[j + 1]
                paired[nodes[j + 1]] = nodes[j]
        return paired

    per_core = []
    for ci in range(N_CORES):
        m = ecore == ci
        sig = defaultdict(lambda: defaultdict(list))
        for s_, g_, sl_ in zip(sources[m], egl[m], eslot[m]):
            sig[s_][g_].append(sl_)
        paired = pair_nodes(sig)
        # collect pair records + residual edges
        pair_recs = []  # (u, v, {g: (slots_u, slots_v)})
        residual = defaultdict(lambda: defaultdict(list))
        done = set()
        for node, gc in sig.items():
            if node in done:
                continue
            v = paired.get(node)
            if v is not None and v not in done:
                done.add(node)
                done.add(v)
                gcv = sig[v]
                rec = {}
                for g in set(gc) | set(gcv):
                    su, sv = gc.get(g, []), gcv.get(g, [])
                    c = min(len(su), len(sv))
                    if c:
                        rec[g] = (su[:c], sv[:c])
                    for sl in su[c:]:
                        residual[node][g].append(sl)
                    for sl in sv[c:]:
                        residual[v][g].append(sl)
                if rec:
                    pair_recs.append((node, v, rec))
            else:
                done.add(node)
                for g, sls in gc.items():
                    residual[node][g].extend(sls)
        # quad merge: match pairs with identical pair signatures
        psig = defaultdict(list)
        for i, (u, v, rec) in enumerate(pair_recs):
            key = tuple(sorted((g, len(su)) for g, (su, sv) in rec.items()))
            psig[key].append(i)
        quad_recs = []
        pair_left = []
        for key, idxlist in psig.items():
            k = len(idxlist) // 2 * 2
            for j in range(0, k, 2):
                quad_recs.append((pair_recs[idxlist[j]], pair_recs[idxlist[j + 1]]))
            if len(idxlist) % 2:
                pair_left.append(pair_recs[idxlist[-1]])
        # second quad-merge pass over leftover pairs
        psig2 = defaultdict(list)
        for i, (u, v, rec) in enumerate(pair_left):
            key = tuple(sorted((g, len(su)) for g, (su, sv) in rec.items()))
            psig2[key].append(i)
        pair_left2 = []
        for key, idxlist in psig2.items():
            k = len(idxlist) // 2 * 2
            for j in range(0, k, 2):
                quad_recs.append((pair_left[idxlist[j]], pair_left[idxlist[j + 1]]))
            if len(idxlist) % 2:
                pair_left2.append(pair_left[idxlist[-1]])
        pair_left = pair_left2
        # octet merge: pair up quads with identical signatures
        qsig = defaultdict(list)
        for i, ((u1, v1, r1), _) in enumerate(quad_recs):
            key = tuple(sorted((g, len(su)) for g, (su, sv) in r1.items()))
            qsig[key].append(i)
        oct_recs = []
        quad_left = []
        for key, idxlist in qsig.items():
            k = len(idxlist) // 2 * 2
            for j in range(0, k, 2):
                oct_recs.append((quad_recs[idxlist[j]], quad_recs[idxlist[j + 1]]))
            if len(idxlist) % 2:
                quad_left.append(quad_recs[idxlist[-1]])
        quad_recs = quad_left
        # residual re-pairing (duplicate rows)
        residual = {n: gc for n, gc in residual.items() if gc}
        paired2 = pair_nodes(residual)
        done2 = set()
        singles = defaultdict(list)  # node -> [(g, slot)]
        for node, gc in residual.items():
            if node in done2:
                continue
            v = paired2.get(node)
            if v is not None and v not in done2:
                done2.add(node)
                done2.add(v)
                gcv = residual[v]
                rec = {}
                for g in set(gc) | set(gcv):
                    su, sv = gc.get(g, []), gcv.get(g, [])
                    c = min(len(su), len(sv))
                    if c:
                        rec[g] = (su[:c], sv[:c])
                    for sl in su[c:]:
                        singles[node].append((g, sl))
                    for sl in sv[c:]:
                        singles[v].append((g, sl))
                if rec:
                    pair_left.append((node, v, rec))
            else:
                done2.add(node)
                for g, sls in gc.items():
                    for sl in sls:
                        singles[node].append((g, sl))
        # merge residual pairs upward as well
        psig3 = defaultdict(list)
        for i, (u, v, rec) in enumerate(pair_left):
            key = tuple(sorted((g, len(su)) for g, (su, sv) in rec.items()))
            psig3[key].append(i)
        pair_left3 = []
        for key, idxlist in psig3.items():
            k = len(idxlist) // 2 * 2
            for j in range(0, k, 2):
                quad_recs.append((pair_left[idxlist[j]], pair_left[idxlist[j + 1]]))
            if len(idxlist) % 2:
                pair_left3.append(pair_left[idxlist[-1]])
        pair_left = pair_left3
        qsig2 = defaultdict(list)
        for i, ((u1, v1, r1), _) in enumerate(quad_recs):
            key = tuple(sorted((g, len(su)) for g, (su, sv) in r1.items()))
            qsig2[key].append(i)
        quad_left2 = []
        for key, idxlist in qsig2.items():
            k = len(idxlist) // 2 * 2
            for j in range(0, k, 2):
                oct_recs.append((quad_recs[idxlist[j]], quad_recs[idxlist[j + 1]]))
            if len(idxlist) % 2:
                quad_left2.append(quad_recs[idxlist[-1]])
        quad_recs = quad_left2
        per_core.append((oct_recs, quad_recs, pair_left, singles))

    # position allocation + per-group row lists
    orow_a = [None] * N_CORES
    qrow_a = [None] * N_CORES
    prow_a = [None] * N_CORES
    srow_a = [None] * N_CORES
    perms = []
    ntab_max = 0
    for ci in range(N_CORES):
        oct_recs, quad_recs, pair_left, singles = per_core[ci]
        perm = []
        orow = [[] for _ in range(gpc)]
        qrow = [[] for _ in range(gpc)]
        prow = [[] for _ in range(gpc)]
        srow = [[] for _ in range(gpc)]
        for ((u11, v11, r11), (u12, v12, r12)), ((u21, v21, r21), (u22, v22, r22)) in oct_recs:
            base = len(perm)
            perm += [u11, v11, u12, v12, u21, v21, u22, v22]
            for g in r11:  # identical signatures across all four pairs
                sls = (r11[g][0], r11[g][1], r12[g][0], r12[g][1],
                       r21[g][0], r21[g][1], r22[g][0], r22[g][1])
                for j in range(len(r11[g][0])):
                    orow[g].append((base,) + tuple(sl[j] for sl in sls))
        for (u1, v1, r1), (u2, v2, r2) in quad_recs:
            base = len(perm)
            perm += [u1, v1, u2, v2]
            for g in r1:  # identical signature -> same groups, same counts
                s1u, s1v = r1[g]
                s2u, s2v = r2[g]
                for j in range(len(s1u)):
                    qrow[g].append((base, s1u[j], s1v[j], s2u[j], s2v[j]))
        # masked quad-merge: any two leftover pair records share a quad tile
        pair_left.sort(
            key=lambda r: tuple(sorted((g, len(su)) for g, (su, sv) in r[2].items()))
        )
        for j2 in range(0, len(pair_left) - 1, 2):
            uA, vA, rA = pair_left[j2]
            uB, vB, rB = pair_left[j2 + 1]
            base = len(perm)
            perm += [uA, vA, uB, vB]
            for g in set(rA) | set(rB):
                suA, svA = rA.get(g, ((), ()))
                suB, svB = rB.get(g, ((), ()))
                for j in range(max(len(suA), len(suB))):
                    qrow[g].append(
                        (
                            base,
                            suA[j] if j < len(suA) else -1,
                            svA[j] if j < len(svA) else -1,
                            suB[j] if j < len(suB) else -1,
                            svB[j] if j < len(svB) else -1,
                        )
                    )
        if len(pair_left) % 2:
            uA, vA, rA = pair_left[-1]
            base = len(perm)
            perm += [uA, vA, uA, uA]
            for g, (su, sv) in rA.items():
                for j in range(len(su)):
                    qrow[g].append((base, su[j], sv[j], -1, -1))
        # force-pair remaining singles per group via fresh duplicate rows
        bygroup = defaultdict(list)
        for node, lst in singles.items():
            for g, sl in lst:
                bygroup[g].append((node, sl))
        for g, lst in bygroup.items():
            for j in range(0, len(lst), 8):
                chunk = lst[j : j + 8]
                base = len(perm)
                perm += [n for n, _ in chunk] + [chunk[0][0]] * (8 - len(chunk))
                orow[g].append(
                    (base,)
                    + tuple(sl for _, sl in chunk)
                    + (-1,) * (8 - len(chunk))
                )
        perms.append(np.array(perm, np.int64))
        ntab_max = max(ntab_max, len(perm))
        orow_a[ci] = orow
        qrow_a[ci] = qrow
        prow_a[ci] = prow
        srow_a[ci] = srow

    ntab = (ntab_max + 16 + 63) // 64 * 64  # + >=8 zero rows at the end
    zrow = ntab - 16
    O = max((len(r) + 127) // 128 for ci in range(N_CORES) for r in orow_a[ci])
    Q = max((len(r) + 127) // 128 for ci in range(N_CORES) for r in qrow_a[ci])
    P = max((len(r) + 127) // 128 for ci in range(N_CORES) for r in prow_a[ci])
    S = max((len(r) + 127) // 128 for ci in range(N_CORES) for r in srow_a[ci])

    idxo = np.full((N_CORES, 128, gpc * O), zrow, np.int32)
    tgto = np.full((N_CORES, 128, gpc * O * 8), -1.0, np.float32)
    idxq = np.full((N_CORES, 128, gpc * Q), zrow, np.int32)
    tgtq = np.full((N_CORES, 128, gpc * Q * 4), -1.0, np.float32)
    idxp = np.full((N_CORES, 128, gpc * P), zrow, np.int32)
    tgtp = np.full((N_CORES, 128, gpc * P * 2), -1.0, np.float32)
    idxs = np.full((N_CORES, 128, gpc * S), zrow, np.int32)
    tgts = np.full((N_CORES, 128, gpc * S), -1.0, np.float32)
    for ci in range(N_CORES):
        for g in range(gpc):
            for r, row in enumerate(orow_a[ci][g]):
                ot, p = r // 128, r % 128
                idxo[ci, p, g * O + ot] = row[0]
                for h in range(8):
                    tgto[ci, p, (g * O + ot) * 8 + h] = row[1 + h]
            for r, (base, s0, s1, s2, s3) in enumerate(qrow_a[ci][g]):
                qt, p = r // 128, r % 128
                idxq[ci, p, g * Q + qt] = base
                for h, sv in enumerate((s0, s1, s2, s3)):
                    tgtq[ci, p, (g * Q + qt) * 4 + h] = sv
            for r, (base, sA, sB) in enumerate(prow_a[ci][g]):
                pt, p = r // 128, r % 128
                idxp[ci, p, g * P + pt] = base
                tgtp[ci, p, (g * P + pt) * 2] = sA
                tgtp[ci, p, (g * P + pt) * 2 + 1] = sB
            for r, (ppos, sl) in enumerate(srow_a[ci][g]):
                st, p = r // 128, r % 128
                idxs[ci, p, g * S + st] = ppos
                tgts[ci, p, g * S + st] = sl

    x_perms = []
    for ci in range(N_CORES):
        xp = np.zeros((ntab, C), ml_dtypes.bfloat16)
        xp[: len(perms[ci])] = x_bf[perms[ci]]
        x_perms.append(xp)

    # repeated-target table: per group O*8 octet sub-cols then Q*4 quad
    # sub-cols, each tgt value repeated 128x along free (bf16)
    nsub = O * 8 + Q * 4
    trep = np.empty((N_CORES, 128, gpc * nsub * 128), ml_dtypes.bfloat16)
    for ci in range(N_CORES):
        cols = np.empty((128, gpc * nsub), np.float32)
        for g in range(gpc):
            cols[:, g * nsub : g * nsub + O * 8] = tgto[ci][:, g * O * 8 : (g + 1) * O * 8]
            cols[:, g * nsub + O * 8 : (g + 1) * nsub] = tgtq[ci][:, g * Q * 4 : (g + 1) * Q * 4]
        trep[ci] = np.broadcast_to(
            cols.astype(ml_dtypes.bfloat16)[:, :, None], (128, gpc * nsub, 128)
        ).reshape(128, gpc * nsub * 128)

    recip = (1.0 / np.maximum(deg, 1)).astype(np.float32)
    recip_tbl = np.ones((N_CORES, 1, gpc * 128), np.float32)
    nodes = np.arange(N_NODES)
    ncore = node_group // gpc
    npos = (node_group % gpc) * 128 + node_slot
    recip_tbl[ncore, 0, npos] = recip
    recip_tbl = np.tile(recip_tbl, (1, 64, 1))

    return (O, Q, P, S, ntab, x_perms, trep, idxo, tgto, idxq, tgtq, idxp,
            tgtp, idxs, tgts, recip_tbl, node_group, node_slot)


def build_nc(gpc, O, Q, P, S, ntab):
    """Build the SPMD bass program (identical on all cores)."""
    f32 = mybir.dt.float32
    bf16 = mybir.dt.bfloat16
    nc = bacc.Bacc("TRN2", num_devices=N_CORES)
    band = gpc * 128

    x_t = nc.dram_tensor("x", [ntab, C], bf16, kind="ExternalInput")
    nsub = O * 8 + Q * 4
    trep_t = nc.dram_tensor("trep", [128, gpc * nsub * 128], bf16, kind="ExternalInput")
    idxo_t = nc.dram_tensor("idxo", [128, gpc * O], mybir.dt.int32, kind="ExternalInput")
    tgto_t = nc.dram_tensor("tgto", [128, gpc * O * 8], f32, kind="ExternalInput")
    idxq_t = nc.dram_tensor("idxq", [128, gpc * Q], mybir.dt.int32, kind="ExternalInput")
    tgtq_t = nc.dram_tensor("tgtq", [128, gpc * Q * 4], f32, kind="ExternalInput")
    if P > 0:
        idxp_t = nc.dram_tensor("idxp", [128, gpc * P], mybir.dt.int32, kind="ExternalInput")
        tgtp_t = nc.dram_tensor("tgtp", [128, gpc * P * 2], f32, kind="ExternalInput")
    if S > 0:
        idxs_t = nc.dram_tensor("idxs", [128, gpc * S], mybir.dt.int32, kind="ExternalInput")
        tgts_t = nc.dram_tensor("tgts", [128, gpc * S], f32, kind="ExternalInput")
    recip_t = nc.dram_tensor("recip", [64, band], f32, kind="ExternalInput")
    iota_t = nc.dram_tensor("iota", [128, 8 * 128], bf16, kind="ExternalInput")
    ones_t = nc.dram_tensor("ones", [1, 128], bf16, kind="ExternalInput")
    gamma_t = nc.dram_tensor("gamma", [64, 1], f32, kind="ExternalInput")
    beta_t = nc.dram_tensor("beta", [64, 1], f32, kind="ExternalInput")
    bvec_t = nc.dram_tensor("bvec", [1, 64], f32, kind="ExternalInput")
    wt_t = nc.dram_tensor("wt", [64, 64], f32, kind="ExternalInput")
    wtb_t = nc.dram_tensor("wtb", [64, 64], bf16, kind="ExternalInput")
    y_t = nc.dram_tensor("y", [band, C], f32, kind="ExternalOutput")

    cc_in = nc.dram_tensor("cc_in", [2, 64], f32, kind="Internal")
    cc_out = nc.dram_tensor("cc_out", [2, 64], f32, kind="Internal", addr_space="Shared")

    eq = mybir.AluOpType.is_equal
    with tile.TileContext(nc) as tc:
        with (
            tc.tile_pool(name="const", bufs=1) as cp,
            tc.tile_pool(name="tbl", bufs=1) as tp,
            tc.tile_pool(name="dst8", bufs=8) as dp8,
            tc.tile_pool(name="dst4", bufs=8) as dp4,
            tc.tile_pool(name="dst2", bufs=8) as dp2,
            tc.tile_pool(name="dst", bufs=8) as dp,
            tc.tile_pool(name="oh", bufs=8) as ohp,
            tc.tile_pool(name="trg", bufs=4) as trgp,
            tc.tile_pool(name="agg", bufs=1) as aggp,
            tc.tile_pool(name="sq", bufs=4) as sqp,
            tc.tile_pool(name="st", bufs=1) as stp,
            tc.tile_pool(name="out", bufs=4) as outp,
            tc.tile_pool(name="pg", bufs=4, space="PSUM") as pgp,
            tc.tile_pool(name="po", bufs=2, space="PSUM") as pop,
            tc.tile_pool(name="pb2", bufs=1, space="PSUM") as pb2p,
        ):
            iota_sb = cp.tile([128, 8 * 128], bf16)
            nc.sync.dma_start(iota_sb[:], iota_t.ap())
            ones_sb = cp.tile([1, 128], bf16)
            nc.sync.dma_start(ones_sb[:], ones_t.ap())
            gamma_sb = cp.tile([64, 1], f32)
            nc.sync.dma_start(gamma_sb[:], gamma_t.ap())
            beta_sb = cp.tile([64, 1], f32)
            nc.sync.dma_start(beta_sb[:], beta_t.ap())
            bvec_sb = cp.tile([1, 64], f32)
            nc.sync.dma_start(bvec_sb[:], bvec_t.ap())
            wt_sb = cp.tile([64, 64], f32)
            nc.sync.dma_start(wt_sb[:], wt_t.ap())
            wtb_sb = cp.tile([64, 64], bf16)
            nc.sync.dma_start(wtb_sb[:], wtb_t.ap())
            recip_sb = cp.tile([64, band], f32)
            nc.sync.dma_start(recip_sb[:], recip_t.ap())
            idxo_sb = tp.tile([128, gpc * O], mybir.dt.int32)
            nc.sync.dma_start(idxo_sb[:], idxo_t.ap())
            tgto_sb = tp.tile([128, gpc * O * 8], f32)
            nc.sync.dma_start(tgto_sb[:], tgto_t.ap())
            idxq_sb = tp.tile([128, gpc * Q], mybir.dt.int32)
            nc.sync.dma_start(idxq_sb[:], idxq_t.ap())
            tgtq_sb = tp.tile([128, gpc * Q * 4], f32)
            nc.sync.dma_start(tgtq_sb[:], tgtq_t.ap())
            if P > 0:
                idxp_sb = tp.tile([128, gpc * P], mybir.dt.int32)
                nc.sync.dma_start(idxp_sb[:], idxp_t.ap())
                tgtp_sb = tp.tile([128, gpc * P * 2], f32)
                nc.sync.dma_start(tgtp_sb[:], tgtp_t.ap())
            if S > 0:
                idxs_sb = tp.tile([128, gpc * S], mybir.dt.int32)
                nc.sync.dma_start(idxs_sb[:], idxs_t.ap())
                tgts_sb = tp.tile([128, gpc * S], f32)
                nc.sync.dma_start(tgts_sb[:], tgts_t.ap())

            aggT = aggp.tile([64, band], bf16)
            sqpart = stp.tile([64, gpc], f32)

            n_mm = 8 * O + 4 * Q + 2 * P + S
            # phase 1: per group, octet/quad/pair/single gathers + matmuls
            for g in range(gpc):
                psum_g = pgp.tile([64, 128], f32, tag="pg")
                trep_g = trgp.tile([128, nsub * 128], bf16, tag="trg")
                nc.sync.dma_start(
                    trep_g[:], trep_t.ap()[:, g * nsub * 128 : (g + 1) * nsub * 128]
                )
                k = 0
                for ot in range(O):
                    col = g * O + ot
                    dst8 = dp8.tile([128, 8 * C], bf16, tag="dst8")
                    nc.gpsimd.indirect_dma_start(
                        out=dst8[:],
                        out_offset=None,
                        in_=x_t.ap(),
                        in_offset=bass.IndirectOffsetOnAxis(
                            ap=idxo_sb[:, col : col + 1], axis=0
                        ),
                    )
                    oh8 = ohp.tile([128, 8 * 128], bf16, tag="oh")
                    nc.vector.tensor_tensor(
                        out=oh8[:],
                        in0=iota_sb[:],
                        in1=trep_g[:, ot * 1024 : (ot + 1) * 1024],
                        op=eq,
                    )
                    for half in range(8):
                        nc.tensor.matmul(
                            out=psum_g[:],
                            lhsT=dst8[:, half * C : (half + 1) * C],
                            rhs=oh8[:, half * 128 : (half + 1) * 128],
                            start=(k == 0),
                            stop=(k == n_mm - 1),
                        )
                        k += 1
                for qt in range(Q):
                    col = g * Q + qt
                    dst4 = dp4.tile([128, 4 * C], bf16, tag="dst4")
                    nc.gpsimd.indirect_dma_start(
                        out=dst4[:],
                        out_offset=None,
                        in_=x_t.ap(),
                        in_offset=bass.IndirectOffsetOnAxis(
                            ap=idxq_sb[:, col : col + 1], axis=0
                        ),
                    )
                    oh4 = ohp.tile([128, 4 * 128], bf16, tag="oh4")
                    nc.vector.tensor_tensor(
                        out=oh4[:],
                        in0=iota_sb[:, : 4 * 128],
                        in1=trep_g[:, O * 1024 + qt * 512 : O * 1024 + (qt + 1) * 512],
                        op=eq,
                    )
                    for half in range(4):
                        nc.tensor.matmul(
                            out=psum_g[:],
                            lhsT=dst4[:, half * C : (half + 1) * C],
                            rhs=oh4[:, half * 128 : (half + 1) * 128],
                            start=(k == 0),
                            stop=(k == n_mm - 1),
                        )
                        k += 1
                for pt in range(P):
                    col = g * P + pt
                    dst2 = dp2.tile([128, 2 * C], bf16, tag="dst2")
                    nc.gpsimd.indirect_dma_start(
                        out=dst2[:],
                        out_offset=None,
                        in_=x_t.ap(),
                        in_offset=bass.IndirectOffsetOnAxis(
                            ap=idxp_sb[:, col : col + 1], axis=0
                        ),
                    )
                    for half in range(2):
                        oh = ohp.tile([128, 128], bf16, tag="oh")
                        nc.vector.tensor_scalar(
                            out=oh[:],
                            in0=iota_sb[:],
                            scalar1=tgtp_sb[:, col * 2 + half : col * 2 + half + 1],
                            scalar2=None,
                            op0=eq,
                        )
                        nc.tensor.matmul(
                            out=psum_g[:],
                            lhsT=dst2[:, half * C : (half + 1) * C],
                            rhs=oh[:],
                            start=(k == 0),
                            stop=(k == n_mm - 1),
                        )
                        k += 1
                for st in range(S):
                    col = g * S + st
                    dst = dp.tile([128, C], bf16, tag="dst")
                    nc.gpsimd.indirect_dma_start(
                        out=dst[:],
                        out_offset=None,
                        in_=x_t.ap(),
                        in_offset=bass.IndirectOffsetOnAxis(
                            ap=idxs_sb[:, col : col + 1], axis=0
                        ),
                    )
                    oh = ohp.tile([128, 128], bf16, tag="oh")
                    nc.vector.tensor_scalar(
                        out=oh[:],
                        in0=iota_sb[:],
                        scalar1=tgts_sb[:, col : col + 1],
                        scalar2=None,
                        op0=eq,
                    )
                    nc.tensor.matmul(
                        out=psum_g[:],
                        lhsT=dst[:],
                        rhs=oh[:],
                        start=(k == 0),
                        stop=(k == n_mm - 1),
                    )
                    k += 1
                nc.vector.tensor_tensor(
                    out=aggT[:, g * 128 : (g + 1) * 128],
                    in0=psum_g[:],
                    in1=recip_sb[:, g * 128 : (g + 1) * 128],
                    op=mybir.AluOpType.mult,
                )
                sq_scr = sqp.tile([64, 128], f32, tag="sq")
                nc.scalar.activation(
                    out=sq_scr[:],
                    in_=aggT[:, g * 128 : (g + 1) * 128],
                    func=mybir.ActivationFunctionType.Square,
                    accum_out=sqpart[:, g : g + 1],
                )

            # BN partial stats -> collective
            s_col = stp.tile([64, 1], f32)
            nc.vector.tensor_reduce(
                out=s_col[:], in_=aggT[:], axis=mybir.AxisListType.X,
                op=mybir.AluOpType.add,
            )
            q_col = stp.tile([64, 1], f32)
            nc.vector.tensor_reduce(
                out=q_col[:], in_=sqpart[:], axis=mybir.AxisListType.X,
                op=mybir.AluOpType.add,
            )
            nc.sync.dma_start(cc_in.ap()[0:1, :], s_col[:, 0:1])
            nc.sync.dma_start(cc_in.ap()[1:2, :], q_col[:, 0:1])
            nc.gpsimd.collective_compute(
                "AllReduce",
                mybir.AluOpType.add,
                ins=[cc_in.ap()],
                outs=[cc_out.ap()],
                replica_groups=[list(range(N_CORES))],
            )
            ssum = stp.tile([64, 1], f32)
            nc.sync.dma_start(ssum[:], cc_out.ap()[0:1, :])
            qsum = stp.tile([64, 1], f32)
            nc.sync.dma_start(qsum[:], cc_out.ap()[1:2, :])

            # BN constants + fold into linear
            inv_n = 1.0 / float(N_NODES)
            mean = stp.tile([64, 1], f32)
            nc.vector.tensor_scalar(
                out=mean[:], in0=ssum[:], scalar1=inv_n, scalar2=None,
                op0=mybir.AluOpType.mult,
            )
            e2 = stp.tile([64, 1], f32)
            nc.vector.tensor_scalar(
                out=e2[:], in0=qsum[:], scalar1=inv_n, scalar2=None,
                op0=mybir.AluOpType.mult,
            )
            m2 = stp.tile([64, 1], f32)
            nc.vector.tensor_tensor(
                out=m2[:], in0=mean[:], in1=mean[:], op=mybir.AluOpType.mult
            )
            var = stp.tile([64, 1], f32)
            nc.vector.tensor_tensor(
                out=var[:], in0=e2[:], in1=m2[:], op=mybir.AluOpType.subtract
            )
            vare = stp.tile([64, 1], f32)
            nc.vector.tensor_scalar(
                out=vare[:], in0=var[:], scalar1=BN_EPS, scalar2=None,
                op0=mybir.AluOpType.add,
            )
            sd = stp.tile([64, 1], f32)
            nc.scalar.activation(
                out=sd[:], in_=vare[:], func=mybir.ActivationFunctionType.Sqrt
            )
            rstd = stp.tile([64, 1], f32)
            nc.vector.reciprocal(out=rstd[:], in_=sd[:])
            a_col = stp.tile([64, 1], f32)
            nc.vector.tensor_tensor(
                out=a_col[:], in0=rstd[:], in1=gamma_sb[:], op=mybir.AluOpType.mult
            )
            w2 = stp.tile([64, 64], mybir.dt.bfloat16)
            nc.vector.tensor_scalar(
                out=w2[:], in0=wt_sb[:], scalar1=a_col[:, 0:1], scalar2=None,
                op0=mybir.AluOpType.mult,
            )
            ma = stp.tile([64, 1], f32)
            nc.vector.tensor_tensor(
                out=ma[:], in0=mean[:], in1=a_col[:], op=mybir.AluOpType.mult
            )
            cvec = stp.tile([64, 1], mybir.dt.bfloat16)
            nc.vector.tensor_tensor(
                out=cvec[:], in0=beta_sb[:], in1=ma[:], op=mybir.AluOpType.subtract
            )
            pb2 = pb2p.tile([1, 64], f32)
            nc.tensor.matmul(out=pb2[:], lhsT=cvec[:], rhs=wtb_sb[:], start=True, stop=True)
            b2 = stp.tile([1, 64], mybir.dt.bfloat16)
            nc.vector.tensor_tensor(
                out=b2[:], in0=pb2[:], in1=bvec_sb[:], op=mybir.AluOpType.add
            )

            # phase 2: out = relu(aggT.T @ W2 + b2)
            for g in range(gpc):
                po = pop.tile([128, 64], f32, tag="po")
                nc.tensor.matmul(
                    out=po[:],
                    lhsT=aggT[:, g * 128 : (g + 1) * 128],
                    rhs=w2[:],
                    start=True,
                    stop=False,
                )
                nc.tensor.matmul(
                    out=po[:], lhsT=ones_sb[:], rhs=b2[:], start=False, stop=True
                )
                ot = outp.tile([128, C], f32, tag="ot")
                nc.scalar.activation(
                    out=ot[:], in_=po[:], func=mybir.ActivationFunctionType.Relu
                )
                nc.sync.dma_start(y_t.ap()[g * 128 : (g + 1) * 128, :], ot[:])

    nc.compile()
    return nc


_CACHE = {}


def _get_nc(gpc, O, Q, P, S, ntab):
    key = (gpc, O, Q, P, S, ntab)
    if key not in _CACHE:
        _CACHE[key] = build_nc(*key)
    return _CACHE[key]


def kernel(x, sources, targets, gamma, beta, W, b, _trace=False):
    x = np.asarray(x, np.float32)
    sources = np.asarray(sources).astype(np.int64)
    targets = np.asarray(targets).astype(np.int64)
    gamma = np.asarray(gamma, np.float32)
    beta = np.asarray(beta, np.float32)
    W = np.asarray(W, np.float32)
    b = np.asarray(b, np.float32)

    gpc = 49
    bf16 = ml_dtypes.bfloat16
    x_bf = x.astype(bf16)
    (O, Q, P, S, ntab, x_perms, trep, idxo, tgto, idxq, tgtq, idxp, tgtp,
     idxs, tgts, recip_tbl, node_group, node_slot) = build_tables(x_bf, sources, targets, gpc)

    iota = np.tile(np.arange(128, dtype=np.float32)[None, :], (128, 8)).astype(bf16)
    ones = np.ones((1, 128), bf16)
    wt = np.ascontiguousarray(W.T)
    in_maps = []
    for i in range(N_CORES):
        in_maps.append(
            {
                "x": x_perms[i],
                "trep": trep[i],
                "idxo": idxo[i],
                "tgto": tgto[i],
                "idxq": idxq[i],
                "tgtq": tgtq[i],
                **({"idxp": idxp[i], "tgtp": tgtp[i]} if P > 0 else {}),
                **({"idxs": idxs[i], "tgts": tgts[i]} if S > 0 else {}),
                "recip": recip_tbl[i],
                "iota": iota,
                "ones": ones,
                "gamma": gamma.reshape(64, 1),
                "beta": beta.reshape(64, 1),
                "bvec": b.reshape(1, 64),
                "wt": wt,
                "wtb": wt.astype(bf16),
            }
        )

    nc = _get_nc(gpc, O, Q, P, S, ntab)
    res = bass_utils.run_bass_kernel_spmd(
        nc, in_maps, core_ids=list(range(N_CORES)), trace=_trace
    )

    out = np.empty((N_NODES, C), np.float32)
    nodes = np.arange(N_NODES)
    ncore = node_group // gpc
    npos = (node_group % gpc) * 128 + node_slot
    for i in range(N_CORES):
        sel = ncore == i
        out[nodes[sel]] = res.results[i]["y"][npos[sel]]
    kernel.last_exec_time_ns = res.exec_time_ns
    return out
